# revision 14
# baseline (speedup 1.0000x reference)
"""Trainium2 Bass kernel for a 2-layer single-head GAT (PyG GATConv style).

v2: like the baseline (graph/data parallel over destination nodes, dense
scatter via one-hot matmul), but phase_D computes the per-edge softmax
weight w[e] = exp(lrelu(a_s[src_e] + a_d[dst_e])) as a [128, chunk] column
block (a_d[dst_e] gathered via a tiny one-hot matmul) and expands it onto
the one-hot dst matrix with a single tensor_scalar per tile, instead of
computing exp over the full dense [128,128] logits. The softmax denominator
rides the main matmul as a constant-1.0 feature column (row col 768).

  - Device, per layer:
      phase A: h||a_s||a_d = x_shard @ [W | W@att_src | W@att_dst]  (bf16)
      AllGather the (h bf16 || 1.0 || a_s fp32) rows -> hx_full [10000, 896]
      phase D: per dst range: dma_gather rows of h[src]; per chunk:
        adg[e] = onehotT_tile @ a_d_range (PE);  z = a_s + adg;
        w = max(exp(z), exp(0.2 z));  Sb_tile = onehot_tile * w[e];
        PSUM[n, 0:769] += Sb^T @ G[:, 0:769]  (col 768 == 1.0 -> denominator)
        epilogue: out = PSUM[:, :768] * (1/PSUM[:, 768]) (+relu for layer 1).
  - Layer 2 input transposed via DMA-transpose (bf16) through DRAM.
"""

import os
import sys
from contextlib import ExitStack

import numpy as np

for _p in ("/opt/trn_rl_repo", "/root/.axon_site/_ro/trn_rl_repo"):
    if os.path.isdir(_p) and _p not in sys.path:
        sys.path.insert(0, _p)

import ml_dtypes  # noqa: E402

import concourse.bass as bass  # noqa: E402
import concourse.tile as tile  # noqa: E402
from concourse import bacc, mybir  # noqa: E402
from concourse.bass_utils import run_bass_kernel_spmd  # noqa: E402
from concourse.masks import make_identity  # noqa: E402

F32 = mybir.dt.float32
BF16 = mybir.dt.bfloat16
I16 = mybir.dt.int16

N_NODES = 10000
DIM = 768
N_CORES = 8
SHARD = N_NODES // N_CORES  # 1250
P = 128
N_RANGES = (SHARD + P - 1) // P  # 10 (last range has 98 nodes)
ROW = 896  # bf16 elems per gathered row (1792B, mult of 256)
ACOL = 770  # a_s stored as fp32 at bf16 cols [770:772]
DEN_COL = 768  # constant 1.0 -> denominator column in the PSUM matmul
NEG_SLOPE = 0.2
CHUNK_T = 8  # edge tiles per dma_gather chunk (1024 idxs; 1536 crashes the ucode)


def _range_rows(r):
    return min(P, SHARD - r * P)


# ---------------------------------------------------------------------------
# host preprocessing
# ---------------------------------------------------------------------------


def preprocess(x, edge_index, W1, att_src1, att_dst1, W2, att_src2, att_dst2):
    """Build per-core input maps + the tile structure (uniform across cores).

    Destination nodes are load-balanced across the 80 (core, range) buckets
    by in-degree so the per-range tile counts (max over cores) carry minimal
    padding; perm_rows maps output row -> node and undoes the permutation.
    """
    n = x.shape[0]
    src = np.concatenate([np.asarray(edge_index[0]), np.arange(n, dtype=np.int64)])
    dst = np.concatenate([np.asarray(edge_index[1]), np.arange(n, dtype=np.int64)])

    # load-balance: snake-deal nodes (sorted by in-degree desc) across the
    # 80 buckets; bucket b = (core b%8, range b//8). Ranges 0-8 hold 128
    # nodes, range 9 holds 98 (matching the 1250-per-core row layout).
    deg = np.bincount(dst, minlength=n)
    order = np.argsort(-deg, kind="stable")
    nbuckets = N_CORES * N_RANGES
    cap = np.array(
        [[P if r < N_RANGES - 1 else SHARD - (N_RANGES - 1) * P
          for r in range(N_RANGES)] for c in range(N_CORES)]
    ).T.reshape(-1)  # bucket b=(r*8+c)
    import heapq

    bucket_nodes = [[] for _ in range(nbuckets)]
    heap = [(0.0, 0, b) for b in range(nbuckets)]
    heapq.heapify(heap)
    for v in order:
        # place in the least-loaded bucket with space
        while True:
            load, cnt, b = heapq.heappop(heap)
            if len(bucket_nodes[b]) < cap[b]:
                break
        bucket_nodes[b].append(v)
        if len(bucket_nodes[b]) < cap[b]:
            heapq.heappush(heap, (load + deg[v], cnt + 1, b))
    node_core = np.zeros(n, dtype=np.int64)
    node_slot = np.zeros(n, dtype=np.int64)  # row within the core (0..1249)
    perm_rows = np.zeros(n, dtype=np.int64)  # output row -> node
    for b in range(nbuckets):
        r, c = b // N_CORES, b % N_CORES
        for j, v in enumerate(bucket_nodes[b]):
            node_core[v] = c
            node_slot[v] = r * P + j
            perm_rows[c * SHARD + r * P + j] = v

    core_of = node_core[dst]
    buckets = [[None] * N_RANGES for _ in range(N_CORES)]
    for c in range(N_CORES):
        sel = core_of == c
        s_c = src[sel]
        d_c = node_slot[dst[sel]]
        order2 = np.argsort(d_c, kind="stable")
        s_c, d_c = s_c[order2], d_c[order2]
        rid = d_c // P
        for r in range(N_RANGES):
            m = rid == r
            buckets[c][r] = (s_c[m], (d_c[m] - r * P).astype(np.int64))

    tiles_per_range = [
        max(
            1,
            max((len(buckets[c][r][0]) + P - 1) // P for c in range(N_CORES)),
        )
        for r in range(N_RANGES)
    ]
    total_tiles = sum(tiles_per_range)
    total_slots = total_tiles * P

    Wv1 = np.concatenate(
        [W1, (W1 @ att_src1)[:, None], (W1 @ att_dst1)[:, None]], axis=1
    ).astype(np.float32)
    Wv2 = np.concatenate(
        [W2, (W2 @ att_src2)[:, None], (W2 @ att_dst2)[:, None]], axis=1
    ).astype(np.float32)

    in_maps = []
    for c in range(N_CORES):
        idx_slots = np.zeros(total_slots, dtype=np.int16)
        rel_slots = np.full(total_slots, -1, dtype=np.int32)
        off = 0
        for r in range(N_RANGES):
            s_r, rel_r = buckets[c][r]
            k = len(s_r)
            rows_r = node_core[s_r] * SHARD + node_slot[s_r]
            idx_slots[off : off + k] = rows_r.astype(np.int16)
            rel_slots[off : off + k] = rel_r
            off += tiles_per_range[r] * P
        # dma_gather index layout: index i -> [partition i%16, slot i//16],
        # replicated across the 8 groups of 16 partitions.
        idx16 = idx_slots.reshape(-1, 16).T  # [16, total_slots/16]
        idx16 = np.tile(idx16, (8, 1)).copy()  # [128, total_slots/16]
        rel = rel_slots.reshape(total_tiles, P)  # [t, p]
        ohf = (rel[:, :, None] == np.arange(P)[None, None, :]).astype(np.float32)
        # oh  [p, t*128+n]: 1.0 where edge slot (t,p) has dst n
        oh = np.ascontiguousarray(ohf.transpose(1, 0, 2).reshape(P, total_tiles * P))
        # ohT [n, t*128+p]: per-tile transpose of oh
        ohT = np.ascontiguousarray(ohf.transpose(2, 0, 1).reshape(P, total_tiles * P))
        xT = np.zeros((DIM, N_RANGES * P), dtype=ml_dtypes.bfloat16)
        own_nodes = perm_rows[c * SHARD : (c + 1) * SHARD]
        xT[:, :SHARD] = np.asarray(x)[own_nodes].T.astype(ml_dtypes.bfloat16)
        in_maps.append(
            {
                "xT": xT,
                "Wv1": Wv1.astype(ml_dtypes.bfloat16),
                "Wv2": Wv2.astype(ml_dtypes.bfloat16),
                "idx": idx16.astype(np.int16),
                "oh": oh.astype(ml_dtypes.bfloat16),
                "ohT": ohT.astype(ml_dtypes.bfloat16),
            }
        )
    return in_maps, tiles_per_range, perm_rows


# ---------------------------------------------------------------------------
# device program
# ---------------------------------------------------------------------------


def build_program(tiles_per_range, repeat=1):
    total_tiles = sum(tiles_per_range)
    total_slots = total_tiles * P

    nc = bacc.Bacc(
        "TRN2",
        target_bir_lowering=False,
        debug=False,
        num_devices=N_CORES,
    )

    xT_d = nc.dram_tensor("xT", [DIM, N_RANGES * P], BF16, kind="ExternalInput")
    Wv1_d = nc.dram_tensor("Wv1", [DIM, DIM + 2], BF16, kind="ExternalInput")
    Wv2_d = nc.dram_tensor("Wv2", [DIM, DIM + 2], BF16, kind="ExternalInput")
    idx_d = nc.dram_tensor("idx", [P, total_slots // 16], I16, kind="ExternalInput")
    oh_d = nc.dram_tensor("oh", [P, total_slots], BF16, kind="ExternalInput")
    ohT_d = nc.dram_tensor("ohT", [P, total_slots], BF16, kind="ExternalInput")
    out_d = nc.dram_tensor("out", [SHARD, DIM], F32, kind="ExternalOutput")

    hx_in = [nc.dram_tensor(f"hx{L}_in", [SHARD, ROW], BF16) for L in (1, 2)]
    hx_full = [
        nc.dram_tensor(f"hx{L}_full", [N_NODES, ROW], BF16, addr_space="Shared")
        for L in (1, 2)
    ]
    h1pad = nc.dram_tensor("h1pad", [N_RANGES * P, DIM], BF16)

    replica_groups = [list(range(N_CORES))]
    KT = DIM // P  # 6 k-tiles

    with tile.TileContext(nc) as tc, ExitStack() as ctx:
        const_p = ctx.enter_context(tc.tile_pool(name="const", bufs=1))
        sb = ctx.enter_context(tc.tile_pool(name="sb", bufs=4))
        sb2 = ctx.enter_context(tc.tile_pool(name="sb2", bufs=4))
        gp = ctx.enter_context(tc.tile_pool(name="gath", bufs=4))
        pp = ctx.enter_context(tc.tile_pool(name="psum", bufs=3, space="PSUM"))
        ppA = ctx.enter_context(tc.tile_pool(name="psumAd", bufs=2, space="PSUM"))

        # resident constants
        idx_sb = const_p.tile([P, total_slots // 16], I16)
        nc.sync.dma_start(out=idx_sb[:], in_=idx_d[:])
        Wv_sb = [
            const_p.tile([P, KT, DIM + 2], BF16, tag="wv0", name="wv0"),
            const_p.tile([P, KT, DIM + 2], BF16, tag="wv1", name="wv1"),
        ]
        for L, wd in enumerate((Wv1_d, Wv2_d)):
            for k in range(KT):
                nc.sync.dma_start(
                    out=Wv_sb[L][:, k, :], in_=wd[k * P : (k + 1) * P, :]
                )
        # a_d per local dst node, bf16 (rhs of the one-hot gather matmul)
        adcol = [const_p.tile([P, N_RANGES], BF16, tag=f"ad{L}", name=f"adcol{L}") for L in (0, 1)]
        adcolf = [const_p.tile([P, N_RANGES], F32, tag=f"adf{L}", name=f"adcolf{L}") for L in (0, 1)]
        asown = [const_p.tile([P, N_RANGES], F32, tag=f"as{L}", name=f"asown{L}") for L in (0, 1)]
        idf = const_p.tile([P, P], F32, tag="idf")
        make_identity(nc, idf[:])
        idb = const_p.tile([P, P], BF16, tag="idb")
        nc.vector.tensor_copy(out=idb[:], in_=idf[:])
        xT_all = const_p.tile([P, KT, N_RANGES * P], BF16)
        nc.sync.dma_start(
            out=xT_all[:], in_=xT_d[:].rearrange("(k p) n -> p k n", p=P)
        )

        # zero the h1pad tail rows once (they feed junk lhsT columns otherwise)
        zpad = const_p.tile([P, DIM], BF16, tag="zpad")
        nc.vector.memset(zpad[:], 0.0)
        nc.sync.dma_start(
            out=h1pad[SHARD : N_RANGES * P, :], in_=zpad[: N_RANGES * P - SHARD, :]
        )

        def phase_A(L, lhsT_tile_fn):
            """node transform: psum[128, 770] = x_tile @ [W | vs | vd]."""
            for nt in range(N_RANGES):
                ps = pp.tile([P, DIM + 4], F32, tag="ps")
                for k in range(KT):
                    lhsT = lhsT_tile_fn(k, nt)
                    rhs = Wv_sb[L][:, k, :]
                    nc.tensor.matmul(
                        out=ps[:, 0:512],
                        lhsT=lhsT,
                        rhs=rhs[:, 0:512],
                        start=(k == 0),
                        stop=(k == KT - 1),
                    )
                    nc.tensor.matmul(
                        out=ps[:, 512 : DIM + 2],
                        lhsT=lhsT,
                        rhs=rhs[:, 512 : DIM + 2],
                        start=(k == 0),
                        stop=(k == KT - 1),
                    )
                rows = _range_rows(nt)
                hxt = sb.tile([P, ROW], BF16, tag="hxt")
                nc.vector.memset(hxt[:, DIM : DIM + 2], 0.0)
                nc.vector.memset(hxt[:, DEN_COL : DEN_COL + 1], 1.0)
                nc.vector.tensor_copy(out=hxt[:, 0:DIM], in_=ps[:, 0:DIM])
                nc.vector.tensor_copy(
                    out=hxt[:, ACOL : ACOL + 2].bitcast(F32),
                    in_=ps[:, DIM : DIM + 1],
                )
                nc.vector.tensor_copy(
                    out=adcol[L][:, nt : nt + 1], in_=ps[:, DIM + 1 : DIM + 2]
                )
                nc.vector.tensor_copy(
                    out=adcolf[L][:, nt : nt + 1], in_=ps[:, DIM + 1 : DIM + 2]
                )
                nc.vector.tensor_copy(
                    out=asown[L][:, nt : nt + 1], in_=ps[:, DIM : DIM + 1]
                )
                nc.sync.dma_start(
                    out=hx_in[L][nt * P : nt * P + rows, 0 : ACOL + 2],
                    in_=hxt[:rows, 0 : ACOL + 2],
                )

        def phase_D(L, epilogue_fn):
            """gather + one-hot-expanded softmax scatter matmul, per dst range."""
            tile_base = 0
            for r in range(N_RANGES):
                T_r = tiles_per_range[r]
                rows = _range_rows(r)
                ps = pp.tile([P, DIM + 4], F32, tag="ps")
                for c0 in range(0, T_r, CHUNK_T):
                    ct = min(CHUNK_T, T_r - c0)
                    slot0 = (tile_base + c0) * P
                    G = gp.tile([P, CHUNK_T, ROW], BF16, tag="G")
                    nc.gpsimd.dma_gather(
                        out_ap=G[:, 0:ct, :],
                        in_ap=hx_full[L][:],
                        idxs_ap=idx_sb[:, slot0 // 16 : (slot0 + ct * P) // 16],
                        num_idxs=ct * P,
                        num_idxs_reg=ct * P,
                        elem_size=ROW,
                    )
                    oh_c = sb2.tile([P, CHUNK_T * P], BF16, tag="ohc")
                    ohT_c = sb2.tile([P, CHUNK_T * P], BF16, tag="ohTc")
                    nc.sync.dma_start(
                        out=oh_c[:, 0 : ct * P], in_=oh_d[:, slot0 : slot0 + ct * P]
                    )
                    nc.sync.dma_start(
                        out=ohT_c[:, 0 : ct * P], in_=ohT_d[:, slot0 : slot0 + ct * P]
                    )
                    # a_d[dst_e] per edge via one-hot matmul
                    adg = ppA.tile([P, CHUNK_T], F32, tag="adg")
                    for i in range(ct):
                        nc.tensor.matmul(
                            out=adg[:, i : i + 1],
                            lhsT=ohT_c[:, i * P : (i + 1) * P],
                            rhs=adcol[L][:, r : r + 1],
                            start=True,
                            stop=True,
                        )
                    z = sb2.tile([P, CHUNK_T], F32, tag="z")
                    e1 = sb2.tile([P, CHUNK_T], F32, tag="e1")
                    w = sb2.tile([P, CHUNK_T], F32, tag="w")
                    a_s = G[:, 0:ct, ACOL : ACOL + 2].bitcast(F32).rearrange(
                        "p c one -> p (c one)"
                    )
                    nc.vector.tensor_tensor(
                        out=z[:, 0:ct], in0=a_s, in1=adg[:, 0:ct],
                        op=mybir.AluOpType.add,
                    )
                    # exp(lrelu(z)) = max(exp(z), exp(0.2 z)); z is O(1) so no
                    # overflow concern (max-free softmax).
                    nc.scalar.activation(
                        out=e1[:, 0:ct], in_=z[:, 0:ct],
                        func=mybir.ActivationFunctionType.Exp,
                    )
                    nc.scalar.activation(
                        out=z[:, 0:ct], in_=z[:, 0:ct],
                        func=mybir.ActivationFunctionType.Exp, scale=NEG_SLOPE,
                    )
                    nc.vector.tensor_tensor(
                        out=w[:, 0:ct], in0=e1[:, 0:ct], in1=z[:, 0:ct],
                        op=mybir.AluOpType.max,
                    )
                    Sb = sb2.tile([P, CHUNK_T * P], BF16, tag="Sb")
                    for i in range(ct):
                        sl = slice(i * P, (i + 1) * P)
                        nc.vector.tensor_scalar(
                            out=Sb[:, sl],
                            in0=oh_c[:, sl],
                            scalar1=w[:, i : i + 1],
                            scalar2=None,
                            op0=mybir.AluOpType.mult,
                        )
                        first = c0 == 0 and i == 0
                        last = c0 + i == T_r - 1
                        nc.tensor.matmul(
                            out=ps[:, 0:512], lhsT=Sb[:, sl], rhs=G[:, i, 0:512],
                            start=first, stop=last,
                        )
                        nc.tensor.matmul(
                            out=ps[:, 512 : DIM + 1],
                            lhsT=Sb[:, sl],
                            rhs=G[:, i, 512 : DIM + 1],
                            start=first, stop=last,
                        )
                epilogue_fn(r, ps)
                tile_base += T_r

        # ---------------- layer 1 ----------------
        for _rep in range(repeat):
            phase_A(0, lambda k, nt: xT_all[:, k, nt * P : (nt + 1) * P])
            nc.gpsimd.collective_compute(
                "AllGather",
                mybir.AluOpType.bypass,
                replica_groups=replica_groups,
                ins=[hx_in[0][:]],
                outs=[hx_full[0][:]],
            )

            def epi1(r, ps):
                rows = _range_rows(r)
                rec = sb.tile([P, 1], F32, tag="rec")
                nc.vector.reciprocal(out=rec[:rows], in_=ps[:rows, DEN_COL : DEN_COL + 1])
                h1t = sb.tile([P, DIM], BF16, tag="h1t")
                nc.scalar.activation(
                    out=h1t[:rows], in_=ps[:rows, 0:DIM],
                    func=mybir.ActivationFunctionType.Relu, scale=rec[:rows],
                )
                nc.sync.dma_start(out=h1pad[r * P : r * P + rows, :], in_=h1t[:rows, :])

            phase_D(0, epi1)

            # ---------------- layer 2 ----------------
            h1T = [const_p.tile([P, N_RANGES * P], BF16, tag=f"h1T{j}", name=f"h1T{j}") for j in range(KT)]
            for j in range(KT):
                nc.sync.dma_start_transpose(
                    out=h1T[j][:], in_=h1pad[:, j * P : (j + 1) * P]
                )
            phase_A(1, lambda k, nt: h1T[k][:, nt * P : (nt + 1) * P])
            nc.gpsimd.collective_compute(
                "AllGather",
                mybir.AluOpType.bypass,
                replica_groups=replica_groups,
                ins=[hx_in[1][:]],
                outs=[hx_full[1][:]],
            )

            def epi2(r, ps):
                rows = _range_rows(r)
                rec = sb.tile([P, 1], F32, tag="rec")
                nc.vector.reciprocal(out=rec[:rows], in_=ps[:rows, DEN_COL : DEN_COL + 1])
                ot = sb.tile([P, DIM], F32, tag="ot")
                nc.scalar.activation(
                    out=ot[:rows], in_=ps[:rows, 0:DIM],
                    func=mybir.ActivationFunctionType.Copy, scale=rec[:rows],
                )
                nc.sync.dma_start(out=out_d[r * P : r * P + rows, :], in_=ot[:rows, :])

            phase_D(1, epi2)

    nc.compile()
    return nc


# ---------------------------------------------------------------------------
# entry point
# ---------------------------------------------------------------------------

_CACHE = {}


def _get_program(tiles_per_range):
    key = tuple(tiles_per_range)
    if key not in _CACHE:
        _CACHE[key] = build_program(tiles_per_range)
    return _CACHE[key]


def kernel(x, edge_index, W1, att_src1, att_dst1, b1, W2, att_src2, att_dst2, b2):
    x = np.asarray(x, dtype=np.float32)
    edge_index = np.asarray(edge_index)
    in_maps, tiles_per_range, perm_rows = preprocess(
        x, edge_index,
        np.asarray(W1, np.float32), np.asarray(att_src1, np.float32),
        np.asarray(att_dst1, np.float32),
        np.asarray(W2, np.float32), np.asarray(att_src2, np.float32),
        np.asarray(att_dst2, np.float32),
    )
    b1 = np.asarray(b1, np.float32)
    b2 = np.asarray(b2, np.float32)
    if np.any(b1):
        raise NotImplementedError("nonzero b1 not supported by this kernel build")
    nc = _get_program(tiles_per_range)
    res = run_bass_kernel_spmd(nc, in_maps, list(range(N_CORES)))
    out = np.concatenate([res.results[c]["out"] for c in range(N_CORES)], axis=0)
    res_full = np.empty_like(out)
    res_full[perm_rows] = out
    return (res_full + b2).astype(np.float32)


# revision 15
# speedup vs baseline: 1.2530x; 1.2530x over previous
"""Trainium2 Bass kernel for a 2-layer single-head GAT (PyG GATConv style).

v2: like the baseline (graph/data parallel over destination nodes, dense
scatter via one-hot matmul), but phase_D computes the per-edge softmax
weight w[e] = exp(lrelu(a_s[src_e] + a_d[dst_e])) as a [128, chunk] column
block (a_d[dst_e] gathered via a tiny one-hot matmul) and expands it onto
the one-hot dst matrix with a single tensor_scalar per tile, instead of
computing exp over the full dense [128,128] logits. The softmax denominator
rides the main matmul as a constant-1.0 feature column (row col 768).

  - Device, per layer:
      phase A: h||a_s||a_d = x_shard @ [W | W@att_src | W@att_dst]  (bf16)
      AllGather the (h bf16 || 1.0 || a_s fp32) rows -> hx_full [10000, 896]
      phase D: per dst range: dma_gather rows of h[src]; per chunk:
        adg[e] = onehotT_tile @ a_d_range (PE);  z = a_s + adg;
        w = max(exp(z), exp(0.2 z));  Sb_tile = onehot_tile * w[e];
        PSUM[n, 0:769] += Sb^T @ G[:, 0:769]  (col 768 == 1.0 -> denominator)
        epilogue: out = PSUM[:, :768] * (1/PSUM[:, 768]) (+relu for layer 1).
  - Layer 2 input transposed via DMA-transpose (bf16) through DRAM.
"""

import os
import sys
from contextlib import ExitStack

import numpy as np

for _p in ("/opt/trn_rl_repo", "/root/.axon_site/_ro/trn_rl_repo"):
    if os.path.isdir(_p) and _p not in sys.path:
        sys.path.insert(0, _p)

import ml_dtypes  # noqa: E402

import concourse.bass as bass  # noqa: E402
import concourse.tile as tile  # noqa: E402
from concourse import bacc, mybir  # noqa: E402
from concourse.bass_utils import run_bass_kernel_spmd  # noqa: E402
from concourse.masks import make_identity  # noqa: E402

F32 = mybir.dt.float32
BF16 = mybir.dt.bfloat16
I16 = mybir.dt.int16

N_NODES = 10000
DIM = 768
N_CORES = 8
SHARD = N_NODES // N_CORES  # 1250
P = 128
N_RANGES = (SHARD + P - 1) // P  # 10 (last range has 98 nodes)
ROW = 896  # bf16 elems per gathered row (1792B, mult of 256)
ACOL = 770  # a_s stored as fp32 at bf16 cols [770:772]
DEN_COL = 768  # constant 1.0 -> denominator column in the PSUM matmul
NEG_SLOPE = 0.2
CHUNK_T = 8  # edge tiles per dma_gather chunk (1024 idxs; 1536 crashes the ucode)


def _range_rows(r):
    return min(P, SHARD - r * P)


# ---------------------------------------------------------------------------
# host preprocessing
# ---------------------------------------------------------------------------


def preprocess(x, edge_index, W1, att_src1, att_dst1, W2, att_src2, att_dst2):
    """Build per-core input maps + the tile structure (uniform across cores).

    Destination nodes are load-balanced across the 80 (core, range) buckets
    by in-degree so the per-range tile counts (max over cores) carry minimal
    padding; perm_rows maps output row -> node and undoes the permutation.
    """
    n = x.shape[0]
    src = np.concatenate([np.asarray(edge_index[0]), np.arange(n, dtype=np.int64)])
    dst = np.concatenate([np.asarray(edge_index[1]), np.arange(n, dtype=np.int64)])

    # load-balance: snake-deal nodes (sorted by in-degree desc) across the
    # 80 buckets; bucket b = (core b%8, range b//8). Ranges 0-8 hold 128
    # nodes, range 9 holds 98 (matching the 1250-per-core row layout).
    deg = np.bincount(dst, minlength=n)
    order = np.argsort(-deg, kind="stable")
    nbuckets = N_CORES * N_RANGES
    cap = np.array(
        [[P if r < N_RANGES - 1 else SHARD - (N_RANGES - 1) * P
          for r in range(N_RANGES)] for c in range(N_CORES)]
    ).T.reshape(-1)  # bucket b=(r*8+c)
    import heapq

    bucket_nodes = [[] for _ in range(nbuckets)]
    heap = [(0.0, 0, b) for b in range(nbuckets)]
    heapq.heapify(heap)
    for v in order:
        # place in the least-loaded bucket with space
        while True:
            load, cnt, b = heapq.heappop(heap)
            if len(bucket_nodes[b]) < cap[b]:
                break
        bucket_nodes[b].append(v)
        if len(bucket_nodes[b]) < cap[b]:
            heapq.heappush(heap, (load + deg[v], cnt + 1, b))
    node_core = np.zeros(n, dtype=np.int64)
    node_slot = np.zeros(n, dtype=np.int64)  # row within the core (0..1249)
    perm_rows = np.zeros(n, dtype=np.int64)  # output row -> node
    for b in range(nbuckets):
        r, c = b // N_CORES, b % N_CORES
        for j, v in enumerate(bucket_nodes[b]):
            node_core[v] = c
            node_slot[v] = r * P + j
            perm_rows[c * SHARD + r * P + j] = v

    core_of = node_core[dst]
    buckets = [[None] * N_RANGES for _ in range(N_CORES)]
    for c in range(N_CORES):
        sel = core_of == c
        s_c = src[sel]
        d_c = node_slot[dst[sel]]
        order2 = np.argsort(d_c, kind="stable")
        s_c, d_c = s_c[order2], d_c[order2]
        rid = d_c // P
        for r in range(N_RANGES):
            m = rid == r
            buckets[c][r] = (s_c[m], (d_c[m] - r * P).astype(np.int64))

    tiles_per_range = [
        max(
            1,
            max((len(buckets[c][r][0]) + P - 1) // P for c in range(N_CORES)),
        )
        for r in range(N_RANGES)
    ]
    total_tiles = sum(tiles_per_range)
    total_slots = total_tiles * P

    Wv1 = np.concatenate(
        [W1, (W1 @ att_src1)[:, None], (W1 @ att_dst1)[:, None]], axis=1
    ).astype(np.float32)
    Wv2 = np.concatenate(
        [W2, (W2 @ att_src2)[:, None], (W2 @ att_dst2)[:, None]], axis=1
    ).astype(np.float32)

    in_maps = []
    for c in range(N_CORES):
        idx_slots = np.zeros(total_slots, dtype=np.int16)
        rel_slots = np.full(total_slots, -1, dtype=np.int32)
        off = 0
        for r in range(N_RANGES):
            s_r, rel_r = buckets[c][r]
            k = len(s_r)
            rows_r = node_core[s_r] * SHARD + node_slot[s_r]
            idx_slots[off : off + k] = rows_r.astype(np.int16)
            rel_slots[off : off + k] = rel_r
            off += tiles_per_range[r] * P
        # dma_gather index layout: index i -> [partition i%16, slot i//16],
        # replicated across the 8 groups of 16 partitions.
        idx16 = idx_slots.reshape(-1, 16).T  # [16, total_slots/16]
        idx16 = np.tile(idx16, (8, 1)).copy()  # [128, total_slots/16]
        rel = rel_slots.reshape(total_tiles, P)  # [t, p]
        ohf = (rel[:, :, None] == np.arange(P)[None, None, :]).astype(np.float32)
        # oh  [p, t*128+n]: 1.0 where edge slot (t,p) has dst n
        oh = np.ascontiguousarray(ohf.transpose(1, 0, 2).reshape(P, total_tiles * P))
        # ohT [n, t*128+p]: per-tile transpose of oh
        ohT = np.ascontiguousarray(ohf.transpose(2, 0, 1).reshape(P, total_tiles * P))
        xT = np.zeros((DIM, N_RANGES * P), dtype=ml_dtypes.bfloat16)
        own_nodes = perm_rows[c * SHARD : (c + 1) * SHARD]
        xT[:, :SHARD] = np.asarray(x)[own_nodes].T.astype(ml_dtypes.bfloat16)
        in_maps.append(
            {
                "xT": xT,
                "Wv1": Wv1.astype(ml_dtypes.bfloat16),
                "Wv2": Wv2.astype(ml_dtypes.bfloat16),
                "idx": idx16.astype(np.int16),
                "oh": oh.astype(ml_dtypes.bfloat16),
                "ohT": ohT.astype(ml_dtypes.bfloat16),
            }
        )
    return in_maps, tiles_per_range, perm_rows


# ---------------------------------------------------------------------------
# device program
# ---------------------------------------------------------------------------


def build_program(tiles_per_range, repeat=1):
    total_tiles = sum(tiles_per_range)
    total_slots = total_tiles * P

    nc = bacc.Bacc(
        "TRN2",
        target_bir_lowering=False,
        debug=False,
        num_devices=N_CORES,
    )

    xT_d = nc.dram_tensor("xT", [DIM, N_RANGES * P], BF16, kind="ExternalInput")
    Wv1_d = nc.dram_tensor("Wv1", [DIM, DIM + 2], BF16, kind="ExternalInput")
    Wv2_d = nc.dram_tensor("Wv2", [DIM, DIM + 2], BF16, kind="ExternalInput")
    idx_d = nc.dram_tensor("idx", [P, total_slots // 16], I16, kind="ExternalInput")
    oh_d = nc.dram_tensor("oh", [P, total_slots], BF16, kind="ExternalInput")
    ohT_d = nc.dram_tensor("ohT", [P, total_slots], BF16, kind="ExternalInput")
    out_d = nc.dram_tensor("out", [SHARD, DIM], F32, kind="ExternalOutput")

    hx_in = [nc.dram_tensor(f"hx{L}_in", [SHARD, ROW], BF16) for L in (1, 2)]
    hx_full = [
        nc.dram_tensor(f"hx{L}_full", [N_NODES, ROW], BF16, addr_space="Shared")
        for L in (1, 2)
    ]
    h1pad = nc.dram_tensor("h1pad", [N_RANGES * P, DIM], BF16)

    replica_groups = [list(range(N_CORES))]
    KT = DIM // P  # 6 k-tiles

    with tile.TileContext(nc) as tc, ExitStack() as ctx:
        const_p = ctx.enter_context(tc.tile_pool(name="const", bufs=1))
        sb = ctx.enter_context(tc.tile_pool(name="sb", bufs=4))
        sb2 = ctx.enter_context(tc.tile_pool(name="sb2", bufs=4))
        gp = ctx.enter_context(tc.tile_pool(name="gath", bufs=6))
        pp = ctx.enter_context(tc.tile_pool(name="psum", bufs=3, space="PSUM"))
        ppA = ctx.enter_context(tc.tile_pool(name="psumAd", bufs=2, space="PSUM"))

        # resident constants
        idx_sb = const_p.tile([P, total_slots // 16], I16)
        nc.sync.dma_start(out=idx_sb[:], in_=idx_d[:])
        Wv_sb = [
            const_p.tile([P, KT, DIM + 2], BF16, tag="wv0", name="wv0"),
            const_p.tile([P, KT, DIM + 2], BF16, tag="wv1", name="wv1"),
        ]
        for L, wd in enumerate((Wv1_d, Wv2_d)):
            for k in range(KT):
                nc.sync.dma_start(
                    out=Wv_sb[L][:, k, :], in_=wd[k * P : (k + 1) * P, :]
                )
        # a_d per local dst node, bf16 (rhs of the one-hot gather matmul)
        adcol = [const_p.tile([P, N_RANGES], BF16, tag=f"ad{L}", name=f"adcol{L}") for L in (0, 1)]
        adcolf = [const_p.tile([P, N_RANGES], F32, tag=f"adf{L}", name=f"adcolf{L}") for L in (0, 1)]
        asown = [const_p.tile([P, N_RANGES], F32, tag=f"as{L}", name=f"asown{L}") for L in (0, 1)]
        idf = const_p.tile([P, P], F32, tag="idf")
        make_identity(nc, idf[:])
        idb = const_p.tile([P, P], BF16, tag="idb")
        nc.vector.tensor_copy(out=idb[:], in_=idf[:])
        xT_all = const_p.tile([P, KT, N_RANGES * P], BF16)
        nc.sync.dma_start(
            out=xT_all[:], in_=xT_d[:].rearrange("(k p) n -> p k n", p=P)
        )

        # zero the h1pad tail rows once (they feed junk lhsT columns otherwise)
        zpad = const_p.tile([P, DIM], BF16, tag="zpad")
        nc.vector.memset(zpad[:], 0.0)
        nc.sync.dma_start(
            out=h1pad[SHARD : N_RANGES * P, :], in_=zpad[: N_RANGES * P - SHARD, :]
        )

        def phase_A(L, lhsT_tile_fn):
            """node transform: psum[128, 770] = x_tile @ [W | vs | vd]."""
            for nt in range(N_RANGES):
                ps = pp.tile([P, DIM + 4], F32, tag="ps")
                for k in range(KT):
                    lhsT = lhsT_tile_fn(k, nt)
                    rhs = Wv_sb[L][:, k, :]
                    nc.tensor.matmul(
                        out=ps[:, 0:512],
                        lhsT=lhsT,
                        rhs=rhs[:, 0:512],
                        start=(k == 0),
                        stop=(k == KT - 1),
                    )
                    nc.tensor.matmul(
                        out=ps[:, 512 : DIM + 2],
                        lhsT=lhsT,
                        rhs=rhs[:, 512 : DIM + 2],
                        start=(k == 0),
                        stop=(k == KT - 1),
                    )
                rows = _range_rows(nt)
                hxt = sb.tile([P, ROW], BF16, tag="hxt")
                nc.vector.memset(hxt[:, DIM : DIM + 2], 0.0)
                nc.vector.memset(hxt[:, DEN_COL : DEN_COL + 1], 1.0)
                nc.vector.tensor_copy(out=hxt[:, 0:DIM], in_=ps[:, 0:DIM])
                nc.vector.tensor_copy(
                    out=hxt[:, ACOL : ACOL + 2].bitcast(F32),
                    in_=ps[:, DIM : DIM + 1],
                )
                nc.vector.tensor_copy(
                    out=adcol[L][:, nt : nt + 1], in_=ps[:, DIM + 1 : DIM + 2]
                )
                nc.vector.tensor_copy(
                    out=adcolf[L][:, nt : nt + 1], in_=ps[:, DIM + 1 : DIM + 2]
                )
                nc.vector.tensor_copy(
                    out=asown[L][:, nt : nt + 1], in_=ps[:, DIM : DIM + 1]
                )
                nc.sync.dma_start(
                    out=hx_in[L][nt * P : nt * P + rows, 0 : ACOL + 2],
                    in_=hxt[:rows, 0 : ACOL + 2],
                )

        def phase_D(L, epilogue_fn):
            """gather + one-hot-expanded softmax scatter matmul, per dst range."""
            tile_base = 0
            for r in range(N_RANGES):
                T_r = tiles_per_range[r]
                rows = _range_rows(r)
                ps = pp.tile([P, DIM + 4], F32, tag="ps")
                for c0 in range(0, T_r, CHUNK_T):
                    ct = min(CHUNK_T, T_r - c0)
                    slot0 = (tile_base + c0) * P
                    G = gp.tile([P, CHUNK_T, ROW], BF16, tag="G")
                    nc.gpsimd.dma_gather(
                        out_ap=G[:, 0:ct, :],
                        in_ap=hx_full[L][:],
                        idxs_ap=idx_sb[:, slot0 // 16 : (slot0 + ct * P) // 16],
                        num_idxs=ct * P,
                        num_idxs_reg=ct * P,
                        elem_size=ROW,
                    )
                    oh_c = sb2.tile([P, CHUNK_T * P], BF16, tag="ohc")
                    ohT_c = sb2.tile([P, CHUNK_T * P], BF16, tag="ohTc")
                    nc.sync.dma_start(
                        out=oh_c[:, 0 : ct * P], in_=oh_d[:, slot0 : slot0 + ct * P]
                    )
                    nc.sync.dma_start(
                        out=ohT_c[:, 0 : ct * P], in_=ohT_d[:, slot0 : slot0 + ct * P]
                    )
                    # a_d[dst_e] per edge via one-hot matmul
                    adg = ppA.tile([P, CHUNK_T], F32, tag="adg")
                    for i in range(ct):
                        nc.tensor.matmul(
                            out=adg[:, i : i + 1],
                            lhsT=ohT_c[:, i * P : (i + 1) * P],
                            rhs=adcol[L][:, r : r + 1],
                            start=True,
                            stop=True,
                        )
                    z = sb2.tile([P, CHUNK_T], F32, tag="z")
                    e1 = sb2.tile([P, CHUNK_T], F32, tag="e1")
                    w = sb2.tile([P, CHUNK_T], F32, tag="w")
                    a_s = G[:, 0:ct, ACOL : ACOL + 2].bitcast(F32).rearrange(
                        "p c one -> p (c one)"
                    )
                    nc.vector.tensor_tensor(
                        out=z[:, 0:ct], in0=a_s, in1=adg[:, 0:ct],
                        op=mybir.AluOpType.add,
                    )
                    # exp(lrelu(z)) = max(exp(z), exp(0.2 z)); z is O(1) so no
                    # overflow concern (max-free softmax).
                    nc.scalar.activation(
                        out=e1[:, 0:ct], in_=z[:, 0:ct],
                        func=mybir.ActivationFunctionType.Exp,
                    )
                    nc.scalar.activation(
                        out=z[:, 0:ct], in_=z[:, 0:ct],
                        func=mybir.ActivationFunctionType.Exp, scale=NEG_SLOPE,
                    )
                    nc.vector.tensor_tensor(
                        out=w[:, 0:ct], in0=e1[:, 0:ct], in1=z[:, 0:ct],
                        op=mybir.AluOpType.max,
                    )
                    Sb = sb2.tile([P, CHUNK_T * P], BF16, tag="Sb")
                    for i in range(ct):
                        sl = slice(i * P, (i + 1) * P)
                        nc.vector.tensor_scalar(
                            out=Sb[:, sl],
                            in0=oh_c[:, sl],
                            scalar1=w[:, i : i + 1],
                            scalar2=None,
                            op0=mybir.AluOpType.mult,
                        )
                        first = c0 == 0 and i == 0
                        last = c0 + i == T_r - 1
                        nc.tensor.matmul(
                            out=ps[:, 0:512], lhsT=Sb[:, sl], rhs=G[:, i, 0:512],
                            start=first, stop=last,
                        )
                        nc.tensor.matmul(
                            out=ps[:, 512 : DIM + 1],
                            lhsT=Sb[:, sl],
                            rhs=G[:, i, 512 : DIM + 1],
                            start=first, stop=last,
                        )
                epilogue_fn(r, ps)
                tile_base += T_r

        # ---------------- layer 1 ----------------
        for _rep in range(repeat):
            phase_A(0, lambda k, nt: xT_all[:, k, nt * P : (nt + 1) * P])
            nc.gpsimd.collective_compute(
                "AllGather",
                mybir.AluOpType.bypass,
                replica_groups=replica_groups,
                ins=[hx_in[0][:]],
                outs=[hx_full[0][:]],
            )

            def epi1(r, ps):
                rows = _range_rows(r)
                rec = sb.tile([P, 1], F32, tag="rec")
                nc.vector.reciprocal(out=rec[:rows], in_=ps[:rows, DEN_COL : DEN_COL + 1])
                h1t = sb.tile([P, DIM], BF16, tag="h1t")
                nc.scalar.activation(
                    out=h1t[:rows], in_=ps[:rows, 0:DIM],
                    func=mybir.ActivationFunctionType.Relu, scale=rec[:rows],
                )
                nc.sync.dma_start(out=h1pad[r * P : r * P + rows, :], in_=h1t[:rows, :])

            phase_D(0, epi1)

            # ---------------- layer 2 ----------------
            h1T = [const_p.tile([P, N_RANGES * P], BF16, tag=f"h1T{j}", name=f"h1T{j}") for j in range(KT)]
            for j in range(KT):
                nc.sync.dma_start_transpose(
                    out=h1T[j][:], in_=h1pad[:, j * P : (j + 1) * P]
                )
            phase_A(1, lambda k, nt: h1T[k][:, nt * P : (nt + 1) * P])
            nc.gpsimd.collective_compute(
                "AllGather",
                mybir.AluOpType.bypass,
                replica_groups=replica_groups,
                ins=[hx_in[1][:]],
                outs=[hx_full[1][:]],
            )

            def epi2(r, ps):
                rows = _range_rows(r)
                rec = sb.tile([P, 1], F32, tag="rec")
                nc.vector.reciprocal(out=rec[:rows], in_=ps[:rows, DEN_COL : DEN_COL + 1])
                ot = sb.tile([P, DIM], F32, tag="ot")
                nc.scalar.activation(
                    out=ot[:rows], in_=ps[:rows, 0:DIM],
                    func=mybir.ActivationFunctionType.Copy, scale=rec[:rows],
                )
                nc.sync.dma_start(out=out_d[r * P : r * P + rows, :], in_=ot[:rows, :])

            phase_D(1, epi2)

    nc.compile()
    return nc


# ---------------------------------------------------------------------------
# entry point
# ---------------------------------------------------------------------------

_CACHE = {}


def _get_program(tiles_per_range):
    key = tuple(tiles_per_range)
    if key not in _CACHE:
        _CACHE[key] = build_program(tiles_per_range)
    return _CACHE[key]


def kernel(x, edge_index, W1, att_src1, att_dst1, b1, W2, att_src2, att_dst2, b2):
    x = np.asarray(x, dtype=np.float32)
    edge_index = np.asarray(edge_index)
    in_maps, tiles_per_range, perm_rows = preprocess(
        x, edge_index,
        np.asarray(W1, np.float32), np.asarray(att_src1, np.float32),
        np.asarray(att_dst1, np.float32),
        np.asarray(W2, np.float32), np.asarray(att_src2, np.float32),
        np.asarray(att_dst2, np.float32),
    )
    b1 = np.asarray(b1, np.float32)
    b2 = np.asarray(b2, np.float32)
    if np.any(b1):
        raise NotImplementedError("nonzero b1 not supported by this kernel build")
    nc = _get_program(tiles_per_range)
    res = run_bass_kernel_spmd(nc, in_maps, list(range(N_CORES)))
    out = np.concatenate([res.results[c]["out"] for c in range(N_CORES)], axis=0)
    res_full = np.empty_like(out)
    res_full[perm_rows] = out
    return (res_full + b2).astype(np.float32)
